# revision 14
# baseline (speedup 1.0000x reference)
"""GCNConv on 8 Trainium2 NeuronCores.

out = D^-1/2 (A + I) D^-1/2 (x @ W.T + b)

Dest-bucket sharding: each core owns 6250 destination nodes and the graph
edges pointing at them; every core computes the full projected-feature
table g' = (dis*x) @ W.T (replicated matmul), then gathers its edges'
source rows with SWDGE dma_gather and segment-reduces them on the DVE.

  - host folds dis = deg^-1/2 into x, so phase A is a pure matmul; the
    bias term dis[d]*b*sum_e dis[col_e] and the self-loop term
    dis[d]^2*h[d] are rank-1/index-only and are added on the host from
    the exported g table.
  - phase A: PE matmul (K=256 split in two), batched ACT psum->fp16
    cast, 4-tile-batched DMA writes of the g table (rearranged DRAM APs
    keep the HWDGE sequencer off the critical path).
  - phase B: per-core dest windows of 128 (one dest per SBUF partition),
    degree-sorted; gather calls merged region-major over balanced groups
    of GG windows (~39 dma_gather calls/core; desc-gen on the Pool Q7
    pair is the dominant cost at ~3ns/edge). int16 gather indices span
    only 32768 table rows, so three overlapping base regions split each
    dest's (ascending) source list. fp16 pairwise tree reduce on DVE
    (2x mode), dis[dest] scale on ACT, store.
  - host: inverse-permute window layout, add bias/self terms.
"""

import numpy as np

N_NODES = 50000
N_EDGES = 1600000
IN_CH = 256
OUT_CH = 128
N_CORES = 8

DPC = N_NODES // N_CORES          # dests per core
WPC = (DPC + 127) // 128          # windows per core
NPAD = ((N_NODES + 127) // 128) * 128   # padded node count
NT = NPAD // 128                  # node tiles for the matmul
GT_ROWS = NPAD + 384
MIDSPLIT = 25024                  # node where the middle zero band sits
G2BASE = 8832
G3BASE = 17664
G1MAX = 32511                     # max node reachable from base 0
G2MIN, G2MAX = 8704, 41343
G3MIN = 17536
G2PAD = 16320
G3PAD = 32640
XSLAB_T = 23                      # node tiles per x slab (23*17 = 391 = NT)
PGROUP = 4                        # node tiles per PSUM bank
DISR_P = 4                        # partitions for the dis row vector
DISR_W = NPAD // DISR_P           # nodes per dis-row partition
GG = 4                            # windows per gather group
CALL_CAP = 16                     # 2048-idx calls: stall-free SWDGE rate (2.04ns/idx measured)


def _row_of(n):
    n = np.asarray(n, dtype=np.int64)
    return n + 128 + 128 * (n >= MIDSPLIT)


def _plan(edge_index):
    """Host-side index preprocessing. Returns per-core gather grids, group
    structure, and the permutation needed to unshard."""
    ei0 = np.asarray(edge_index[0], dtype=np.int64)
    ei1 = np.asarray(edge_index[1], dtype=np.int64)
    self_idx = np.arange(N_NODES, dtype=np.int64)
    # degree and dis INCLUDE the self loop (matches the reference)
    deg_full = np.bincount(np.concatenate([ei0, self_idx]),
                           minlength=N_NODES)
    dis = deg_full.astype(np.float32) ** -0.5

    # the gather grid covers only graph edges; the self term is added on
    # the host from the g table
    row = ei0
    col = ei1
    deg = np.bincount(row, minlength=N_NODES)
    n1o = np.bincount(row[col < G2MIN], minlength=N_NODES)
    n3o = np.bincount(row[col > G2MAX], minlength=N_NODES)
    m1 = np.bincount(row[col <= G1MAX], minlength=N_NODES)
    m3 = np.bincount(row[col >= G3MIN], minlength=N_NODES)

    k1 = np.clip((deg + 2) // 3, n1o, m1)
    k3 = np.clip((deg - k1 + 1) // 2, n3o, np.minimum(m3, deg - k1))
    k2 = deg - k1 - k3

    order = np.lexsort((col, row))
    col_sorted = col[order].astype(np.int64)
    row_sorted = _row_of(col_sorted)  # table rows, per-dest ascending
    starts = np.zeros(N_NODES + 1, dtype=np.int64)
    np.cumsum(deg, out=starts[1:])

    maxk = np.maximum(np.maximum(k1, k2), k3)
    perms = []
    kmax_pc = np.zeros((3, N_CORES, WPC), dtype=np.int64)
    for c in range(N_CORES):
        sl = slice(c * DPC, (c + 1) * DPC)
        perm = np.lexsort((-deg[sl], -maxk[sl]))
        perms.append(perm)
        for j, kk in enumerate((k1, k2, k3)):
            ks = kk[sl][perm]
            for w in range(WPC):
                s = w * 128
                e = min(s + 128, DPC)
                kmax_pc[j, c, w] = ks[s:e].max() if s < DPC else 0
    cks = kmax_pc.max(axis=1)          # [3, WPC] shared across cores
    cw = cks.sum(axis=0)

    # balanced groups of GG windows (snake over the width-sorted windows);
    # windows are already in decreasing-degree order so snake assignment
    # equalizes group column sums.
    worder = np.argsort(-cw, kind="stable")
    ngroups = (WPC + GG - 1) // GG
    groups = [[] for _ in range(ngroups)]
    for i, w in enumerate(worder):
        r = i // ngroups
        g = i % ngroups if r % 2 == 0 else ngroups - 1 - (i % ngroups)
        groups[g].append(int(w))
    groups = [sorted(g) for g in groups]

    # region-major column layout per group: for group G the columns are
    # [G1 slabs of its windows | G2 slabs | G3 slabs]; slab offsets are
    # recorded per window (local to the group's base).
    gmeta = []      # per group: dict(base, width, calls, windows)
    totc = 0
    for g in groups:
        base = totc
        calls = []   # (region_idx, global_start_col, ncols)
        slabs = {}   # w -> [(local_off, c_r) for r in 0..2]
        for w in g:
            slabs[w] = []
        off = 0
        for r in range(3):
            rstart = off
            for w in g:
                c_r = int(cks[r, w])
                slabs[w].append((off, c_r))
                off += c_r
            ncols = off - rstart
            s = 0
            while s < ncols:
                cc = min(CALL_CAP, ncols - s)
                calls.append((r, base + rstart + s, cc))
                s += cc
        gmeta.append({
            "base": base, "width": off, "calls": calls,
            "windows": g, "slabs": slabs,
        })
        totc += off
    cmaxg = max(m["width"] for m in gmeta)

    idx_arrs, disw_arrs, gdests = [], [], []
    for c in range(N_CORES):
        perm = perms[c]
        grid = np.zeros((128, totc), dtype=np.int16)
        disw = np.zeros((128, WPC), dtype=np.float32)
        gdest = np.full((WPC, 128), -1, dtype=np.int64)
        for m in gmeta:
            for w in m["windows"]:
                (o1, c1), (o2, c2), (o3, c3) = m["slabs"][w]
                o1 += m["base"]; o2 += m["base"]; o3 += m["base"]
                grid[:, o2:o2 + c2] = G2PAD
                grid[:, o3:o3 + c3] = G3PAD
                for p in range(128):
                    s = w * 128 + p
                    if s >= DPC:
                        break
                    d = c * DPC + int(perm[s])
                    a1, a2, a3 = int(k1[d]), int(k2[d]), int(k3[d])
                    st = int(starts[d])
                    rows_d = row_sorted[st:st + a1 + a2 + a3]
                    grid[p, o1:o1 + a1] = rows_d[:a1].astype(np.int16)
                    grid[p, o2:o2 + a2] = \
                        (rows_d[a1:a1 + a2] - G2BASE).astype(np.int16)
                    grid[p, o3:o3 + a3] = \
                        (rows_d[a1 + a2:] - G3BASE).astype(np.int16)
                    disw[p, w] = dis[d]
                    gdest[w, p] = d
        idx_arrs.append(grid)
        disw_arrs.append(disw)
        gdests.append(gdest)

    idx16 = []
    tot16 = totc * 8
    for c in range(N_CORES):
        L = idx_arrs[c].T.ravel()
        base = L.reshape(tot16, 16).T
        idx16.append(np.ascontiguousarray(np.tile(base, (8, 1))))

    dis_colsum = np.bincount(row, weights=dis[col].astype(np.float64),
                             minlength=N_NODES).astype(np.float32)

    return {
        "dis": dis,
        "dis_colsum": dis_colsum,
        "gmeta": gmeta,
        "cmaxg": cmaxg,
        "totc": totc,
        "tot16": tot16,
        "idx16": idx16,
        "disw": disw_arrs,
        "gdest": gdests,
    }


def _build_bass(gmeta, cmaxg, totc, tot16):
    """Build the single SPMD Bass program (same NEFF on all 8 cores)."""
    import concourse.bacc as bacc
    import concourse.mybir as mybir
    import concourse.tile as tile
    from concourse.library_config import mlp

    fp32 = mybir.dt.float32
    fp16 = mybir.dt.float16
    i16 = mybir.dt.int16

    nc = bacc.Bacc(
        "TRN2",
        target_bir_lowering=False,
        dynamic_dma_scratch_size=65536,
        num_swdge_queues=4,
    )

    xT = nc.dram_tensor("xT", [IN_CH, NPAD], fp16, kind="ExternalInput")
    wT = nc.dram_tensor("wT", [IN_CH, OUT_CH], fp16, kind="ExternalInput")
    disw = nc.dram_tensor("disw", [128, WPC], fp32, kind="ExternalInput")
    idx = nc.dram_tensor("idx", [128, tot16], i16, kind="ExternalInput")

    gtabA = nc.dram_tensor("gtabA", [32768, OUT_CH], fp16,
                           kind="ExternalOutput")
    gtabB = nc.dram_tensor("gtabB", [32768, OUT_CH], fp16, kind="Internal")
    gtabC = nc.dram_tensor("gtabC", [32768, OUT_CH], fp16,
                           kind="ExternalOutput")
    outd = nc.dram_tensor("outd", [WPC, 128, OUT_CH], fp32,
                          kind="ExternalOutput")
    gtab = (gtabA, gtabB, gtabC)

    RBASE = (0, G2BASE, G3BASE)

    with tile.TileContext(nc) as tc:
        with tc.tile_pool(name="globals", bufs=1) as glob:
            # phase-B inputs loaded up front so they never gate the gathers
            dw = glob.tile([128, WPC], fp32, tag="dw")
            nc.sync.dma_start(dw[:], disw[:])
            ix = glob.tile([128, tot16], i16, tag="ix")
            nc.sync.dma_start(ix[:], idx[:])
            self_build(nc, tc, tile, mybir, mlp, gmeta, cmaxg,
                       xT, wT, gtab, outd, dw, ix)
    nc.compile()
    return nc


def self_build(nc, tc, tile, mybir, mlp, gmeta, cmaxg,
               xT, wT, gtab, outd, dw, ix):
    fp32 = mybir.dt.float32
    fp16 = mybir.dt.float16
    RBASE = (0, G2BASE, G3BASE)
    if True:
        # ---------------- phase A: g = x_s @ W.T ----------------
        with (
            tc.tile_pool(name="constA", bufs=1) as cpool,
            tc.tile_pool(name="xslab", bufs=2) as xpool,
            tc.tile_pool(name="gout", bufs=4) as gpool,
            tc.tile_pool(name="psum", bufs=4, space="PSUM") as ppool,
        ):
            nc.gpsimd.load_library(mlp)
            wt0 = cpool.tile([128, OUT_CH], fp16, tag="wt0")
            wt1 = cpool.tile([128, OUT_CH], fp16, tag="wt1")
            nc.sync.dma_start(wt0[:], wT[0:128, :])
            nc.sync.dma_start(wt1[:], wT[128:256, :])

            # zero rows absorb padding gathers (per covering region copy)
            zt = cpool.tile([128, 3 * OUT_CH], fp16, tag="zt")
            nc.vector.memset(zt[:], 0.0)
            for bi, b0 in enumerate((0, MIDSPLIT + 128, NPAD + 256)):
                for ri, gb in enumerate(RBASE):
                    if b0 >= gb and b0 + 128 <= gb + 32768:
                        nc.sync.dma_start(
                            gtab[ri][b0 - gb:b0 - gb + 128, :],
                            zt[:, bi * OUT_CH:(bi + 1) * OUT_CH])

            nslab = NT // XSLAB_T
            for s in range(nslab):
                c0 = s * XSLAB_T * 128
                cols = XSLAB_T * 128
                xa = xpool.tile([128, cols], fp16, tag="xa")
                xb = xpool.tile([128, cols], fp16, tag="xb")
                nc.sync.dma_start(xa[:], xT[0:128, c0:c0 + cols])
                nc.sync.dma_start(xb[:], xT[128:256, c0:c0 + cols])
                t = 0
                while t < XSLAB_T:
                    gn = min(PGROUP, XSLAB_T - t)
                    ps = ppool.tile([128, PGROUP, OUT_CH], fp32, tag="ps")
                    gt = gpool.tile([128, PGROUP, OUT_CH], fp16, tag="gt")
                    for j in range(gn):
                        gt_i = s * XSLAB_T + t + j
                        sl = slice((t + j) * 128, (t + j + 1) * 128)
                        nc.tensor.matmul(
                            ps[:, j, :], xa[:, sl], wt0[:],
                            start=True, stop=False,
                        )
                        nc.tensor.matmul(
                            ps[:, j, :], xb[:, sl], wt1[:],
                            start=False, stop=True,
                        )
                    nc.scalar.activation(
                        gt[:, 0:gn, :], ps[:, 0:gn, :],
                        mybir.ActivationFunctionType.Copy,
                    )
                    t0i = s * XSLAB_T + t
                    rows = [int(_row_of(i * 128)) for i in
                            range(t0i, t0i + gn)]
                    contig = all(
                        int(_row_of(i * 128 + 64)) == int(_row_of(i * 128)) + 64
                        for i in range(t0i, t0i + gn)
                    ) and all(rows[j + 1] == rows[j] + 128
                              for j in range(gn - 1))
                    if contig:
                        r0 = rows[0]
                        for ri, gb in enumerate(RBASE):
                            lo = max(r0, gb)
                            hi = min(r0 + gn * 128, gb + 32768)
                            if lo < hi:
                                jlo = (lo - r0) // 128
                                jhi = (hi - r0) // 128
                                nc.sync.dma_start(
                                    gtab[ri][lo - gb:hi - gb, :].rearrange(
                                        "(j p) c -> p j c", p=128),
                                    gt[:, jlo:jhi, :],
                                )
                    else:
                        for j in range(gn):
                            gt_i = t0i + j
                            r0 = int(_row_of(gt_i * 128))
                            r1 = int(_row_of(gt_i * 128 + 64))
                            halves = ([(r0, 0, 128)] if r1 == r0 + 64 else
                                      [(r0, 0, 64), (r1, 64, 128)])
                            for (rh, p0, p1) in halves:
                                nr = p1 - p0
                                for ri, gb in enumerate(RBASE):
                                    if rh >= gb and rh + nr <= gb + 32768:
                                        nc.sync.dma_start(
                                            gtab[ri][rh - gb:rh - gb + nr, :],
                                            gt[p0:p1, j, :],
                                        )
                    t += gn

        # ---------------- phase B: gather + segment reduce ----------------
        with (
            tc.tile_pool(name="msg", bufs=2) as mpool,
            tc.tile_pool(name="accp", bufs=2) as apool,
            tc.tile_pool(name="red", bufs=4) as rpool,
        ):
            gq = 0
            add = mybir.AluOpType.add

            for m in gmeta:
                base = m["base"]
                width = m["width"]
                msg = mpool.tile([128, cmaxg, OUT_CH], fp16, tag="msg")
                for (r, gstart, cc) in m["calls"]:
                    loc = gstart - base
                    nc.gpsimd.dma_gather(
                        msg[:, loc:loc + cc, :],
                        gtab[r][0:32768, :],
                        ix[:, gstart * 8:(gstart + cc) * 8],
                        128 * cc, 128 * cc, OUT_CH,
                        queue_num=gq % 4,
                        single_packet=False,
                    )
                    gq += 1
                for w in m["windows"]:
                    slabs = [(o, c) for (o, c) in m["slabs"][w] if c > 0]
                    acc = apool.tile([128, cmaxg // 2 + 3, OUT_CH], fp16,
                                     tag="acc")
                    # L1: pairwise-add each slab into contiguous acc cols
                    ao = 0
                    singles = []
                    for (o, c_r) in slabs:
                        h = c_r // 2
                        if h > 0:
                            nc.vector.tensor_tensor(
                                acc[:, ao:ao + h, :],
                                msg[:, o:o + h, :],
                                msg[:, o + h:o + 2 * h, :], op=add,
                            )
                            ao += h
                        if c_r & 1:
                            singles.append(o + 2 * h)
                    for sc in singles:
                        nc.vector.tensor_tensor(
                            acc[:, 0:1, :], acc[:, 0:1, :],
                            msg[:, sc:sc + 1, :], op=add,
                        )
                    c = ao
                    while c > 3:
                        h = c // 2
                        nc.vector.tensor_tensor(
                            acc[:, 0:h, :], acc[:, 0:h, :],
                            acc[:, h:2 * h, :], op=add,
                        )
                        if c & 1:
                            nc.vector.tensor_tensor(
                                acc[:, 0:1, :], acc[:, 0:1, :],
                                acc[:, 2 * h:2 * h + 1, :], op=add,
                            )
                        c = h
                    if c == 3:
                        nc.vector.tensor_tensor(
                            acc[:, 0:1, :], acc[:, 0:1, :], acc[:, 2:3, :],
                            op=add,
                        )
                        c = 2
                    rt = rpool.tile([128, OUT_CH], fp32, tag="rt")
                    if c == 2:
                        nc.vector.tensor_tensor(
                            rt[:], acc[:, 0, :], acc[:, 1, :], op=add,
                        )
                    else:
                        nc.vector.tensor_copy(rt[:], acc[:, 0, :])
                    rt2 = rpool.tile([128, OUT_CH], fp32, tag="rt2")
                    nc.scalar.activation(
                        rt2[:], rt[:], mybir.ActivationFunctionType.Copy,
                        scale=dw[:, w:w + 1],
                    )
                    nc.sync.dma_start(outd[w], rt2[:])


def _install_ntff_shim():
    import sys
    import types
    try:
        import antenv.axon_hooks  # noqa: F401
        return
    except ImportError:
        pass
    hook = None
    try:
        from trn_agent_boot.trn_boot import _ntff_profile_via_ctypes
        hook = _ntff_profile_via_ctypes("/opt/axon/libaxon_pjrt.so")
    except Exception:
        hook = None
    mod = types.ModuleType("antenv.axon_hooks")
    mod._hook = hook
    mod.get_axon_ntff_profile_hook = lambda: mod._hook
    def _set(h):
        mod._hook = h
    mod.set_axon_ntff_profile_hook = _set
    sys.modules["antenv.axon_hooks"] = mod
    try:
        import antenv
        antenv.axon_hooks = mod
    except Exception:
        pass


def kernel(x, edge_index, W, b):
    import os
    os.environ.setdefault("NEURON_RT_RESET_CORES", "1")
    x = np.asarray(x, dtype=np.float32)
    W = np.asarray(W, dtype=np.float32)
    b = np.asarray(b, dtype=np.float32)

    plan = _plan(edge_index)
    nc = _build_bass(plan["gmeta"], plan["cmaxg"], plan["totc"],
                     plan["tot16"])

    xs = x * plan["dis"][:, None]
    xT_pad = np.zeros((IN_CH, NPAD), dtype=np.float16)
    xT_pad[:, :N_NODES] = xs.T.astype(np.float16)
    wT = np.ascontiguousarray(W.T.astype(np.float16))

    in_maps = []
    for c in range(N_CORES):
        in_maps.append({
            "xT": xT_pad,
            "wT": wT,
            "disw": np.ascontiguousarray(plan["disw"][c]),
            "idx": plan["idx16"][c],
        })

    _install_ntff_shim()
    from concourse.bass_utils import run_bass_kernel_spmd
    res = run_bass_kernel_spmd(nc, in_maps, core_ids=list(range(N_CORES)))
    globals()["_last_results"] = res

    out = np.empty((N_NODES, OUT_CH), dtype=np.float32)
    for c in range(N_CORES):
        outd = res.results[c]["outd"]
        gdest = plan["gdest"][c]
        mask = gdest >= 0
        out[gdest[mask]] = outd[mask]

    # host-side terms: self loop dis^2*h[d] and the bias contribution
    # dis[d]*b*sum_e dis[col_e]; the device table g' = dis*(x@W.T) has no
    # bias, so out_dev[d] = dis[d]*sum_e g'[col_e].
    dis = plan["dis"]
    gtA = np.asarray(res.results[0]["gtabA"], dtype=np.float32)
    gtC = np.asarray(res.results[0]["gtabC"], dtype=np.float32)
    rows_n = np.asarray(_row_of(np.arange(N_NODES)))
    lowm = np.arange(N_NODES) < MIDSPLIT
    g_rows = np.empty((N_NODES, OUT_CH), dtype=np.float32)
    g_rows[lowm] = gtA[rows_n[lowm]]
    g_rows[~lowm] = gtC[rows_n[~lowm] - G3BASE]
    bcoef = dis * plan["dis_colsum"] + dis * dis
    out += dis[:, None] * g_rows + bcoef[:, None] * b[None, :]
    return out
